# revision 7
# baseline (speedup 1.0000x reference)
"""Grok1-style GQA attention (S=2048, H=6144, 48 Q heads / 8 KV heads, rope,
softcap-30, causal) as a Bass/Tile kernel sharded over 8 NeuronCores.

Sharding: tensor-parallel across heads. Core c owns Q heads 6c..6c+5 and KV
head c. Each core computes its qkv projection slice, rope, causal softcap
attention for its 6 Q heads against its single KV head, and a partial
o_proj (its 768 columns of w_o). The host sums the 8 partial outputs.

Key numerics trick: softcap bounds scores to [-30, 30], so softmax is
computed as exp(30*tanh(s/30) - 30) with a *constant* bias — no running max.

Schedule: fully software-pipelined so the tensor engine never idles.
  A : qkv(0), hb-major over ob pairs so matmuls start as ht tiles land
  B0: attn(0) interleaved with qkv(1) matmuls   (+ ht(1) prefetch)
  B1: attn(1) interleaved with qkv(2)           (+ ht(2) prefetch)
  B2: attn(2) interleaved with qkv(3)           (+ ht(3) prefetch + wo(0..2))
  C : attn(3) interleaved with o_proj rows 0..11
  D : o_proj rows 12..15 (wo tiles for mc 8..11 still resident from C)
Within attention, score matmuls run 2 iterations ahead of the PV/rowsum
matmuls so the tanh->exp scalar chain never stalls the in-order PE queue;
per-head softmax normalization is deferred one head so the reciprocal/
broadcast chain never blocks the vector engine's triu masking.

Layouts (host-prepped, all transposed so the contraction dim is on SBUF
partitions):
  ht   [4,48,128,512] bf16  : ht[sc,hb,p,c] = hidden[sc*512+c, hb*128+p]
  wq   [8,128,48,128] bf16  : wq[ob,p,hb,o] = w_qkv_core[ob*128+o, hb*128+p]
  wo   [12,128,6,512] bf16  : wo[mc,p,fb,m] = (w_o[:,core]*MULT).T[fb*128+p, mc*512+m]
  cosf/sinf [128,2048] bf16 : duplicated/sign-flipped rope tables (neox)
  triu [128,128] bf16       : triu[k,q] = 1 if q >= k else 0
"""

import sys, os
import numpy as np

sys.path.insert(0, "/opt/trn_rl_repo")

import ml_dtypes

import concourse.bass as bass
import concourse.mybir as mybir
import concourse.tile as tile
from concourse import bacc
from concourse.bass_utils import run_bass_kernel_spmd

F32 = mybir.dt.float32
BF16 = mybir.dt.bfloat16
AF = mybir.ActivationFunctionType

S = 2048
HID = 6144
D = 128
NQ = 6          # q heads per core
N_CORES = 8
SCALE = D ** -0.5
SOFTCAP = 30.0
ATTN_MULT = 0.08838834764831845
ROPE_THETA = 10000.0

N_SC = 4        # s-chunks of 512
SCW = 512
N_HB = 48       # hidden 128-blocks
N_OB = 8        # output 128-blocks per core (6 Q | 1 K | 1 V)
N_MC = 12       # o_proj 512-col chunks
N_SB = 16       # s 128-blocks
N_FB = 6        # per-core o_proj feature 128-blocks (768/128)

OB_ORDER = [6, 7, 0, 1, 2, 3, 4, 5]   # K,V first so next phase never waits


def build_nc():
    nc = bacc.Bacc("TRN2", target_bir_lowering=False, debug=False, num_devices=N_CORES)

    ht_d = nc.dram_tensor("ht", [N_SC, N_HB, 128, SCW], BF16, kind="ExternalInput").ap()
    wq_d = nc.dram_tensor("wq", [N_OB, 128, N_HB, 128], BF16, kind="ExternalInput").ap()
    wo_d = nc.dram_tensor("wo", [N_MC, 128, N_FB, SCW], BF16, kind="ExternalInput").ap()
    cosf_d = nc.dram_tensor("cosf", [128, S], BF16, kind="ExternalInput").ap()
    sinf_d = nc.dram_tensor("sinf", [128, S], BF16, kind="ExternalInput").ap()
    triu_d = nc.dram_tensor("triu", [128, 128], BF16, kind="ExternalInput").ap()
    ones_col_d = nc.dram_tensor("ones_col", [128, 1], BF16, kind="ExternalInput").ap()
    ident_d = nc.dram_tensor("ident", [128, 128], BF16, kind="ExternalInput").ap()
    negcap_d = nc.dram_tensor("negcap", [128, 1], F32, kind="ExternalInput").ap()
    out_d = nc.dram_tensor("out", [S, HID], BF16, kind="ExternalOutput").ap()

    from contextlib import ExitStack
    with tile.TileContext(nc) as tc, ExitStack() as ctx:
        const = ctx.enter_context(tc.tile_pool(name="const", bufs=1))
        ktp = ctx.enter_context(tc.tile_pool(name="ktp", bufs=4))
        vnp = ctx.enter_context(tc.tile_pool(name="vnp", bufs=4))
        aotp = ctx.enter_context(tc.tile_pool(name="aotp", bufs=24))
        qtp = ctx.enter_context(tc.tile_pool(name="qtp", bufs=12))
        vtp = ctx.enter_context(tc.tile_pool(name="vtp", bufs=2))
        htp = ctx.enter_context(tc.tile_pool(name="htp", bufs=50))
        wqp = ctx.enter_context(tc.tile_pool(name="wqp", bufs=4))
        ropep = ctx.enter_context(tc.tile_pool(name="ropep", bufs=2))
        tpool = ctx.enter_context(tc.tile_pool(name="tpool", bufs=2))
        ppool = ctx.enter_context(tc.tile_pool(name="ppool", bufs=4))
        rpool = ctx.enter_context(tc.tile_pool(name="rpool", bufs=2))
        bpool = ctx.enter_context(tc.tile_pool(name="bpool", bufs=2))
        wop = ctx.enter_context(tc.tile_pool(name="wop", bufs=4))
        outp = ctx.enter_context(tc.tile_pool(name="outp", bufs=3))
        ps_a = ctx.enter_context(tc.tile_pool(name="ps_a", bufs=2, space=bass.MemorySpace.PSUM))
        ps_s = ctx.enter_context(tc.tile_pool(name="ps_s", bufs=2, space=bass.MemorySpace.PSUM))
        ps_pv = ctx.enter_context(tc.tile_pool(name="ps_pv", bufs=2, space=bass.MemorySpace.PSUM))
        ps_o = ctx.enter_context(tc.tile_pool(name="ps_o", bufs=2, space=bass.MemorySpace.PSUM))

        # ---- weight prefetch for qkv(0) first pair, via fast HWDGE ----
        wq_pref = {}

        def load_wq_sp8(sc_ob):
            ob = sc_ob[1]
            w_sb = wqp.tile([128, N_HB * 128], BF16, tag="wq", name="wq")
            for qd in range(8):
                nc.sync.dma_start(
                    w_sb[:, qd * 6 * 128:(qd + 1) * 6 * 128],
                    wq_d[ob, :, qd * 6:(qd + 1) * 6])
            wq_pref[sc_ob] = w_sb

        def load_wq_gp4(sc_ob):
            ob = sc_ob[1]
            w_sb = wqp.tile([128, N_HB * 128], BF16, tag="wq", name="wq")
            for qd in range(4):
                nc.gpsimd.dma_start(
                    w_sb[:, qd * 12 * 128:(qd + 1) * 12 * 128],
                    wq_d[ob, :, qd * 12:(qd + 1) * 12])
            wq_pref[sc_ob] = w_sb

        load_wq_sp8((0, 0))

        cosf = const.tile([128, S], BF16, tag="cosf", name="cosf")
        sinf = const.tile([128, S], BF16, tag="sinf", name="sinf")
        triu = const.tile([128, 128], BF16, tag="triu", name="triu")
        ones_col = const.tile([128, 1], BF16, tag="ones_col", name="ones_col")
        ident = const.tile([128, 128], BF16, tag="ident", name="ident")
        negcap = const.tile([128, 1], F32, tag="negcap", name="negcap")
        nc.scalar.dma_start(cosf[:], cosf_d[:])
        nc.scalar.dma_start(sinf[:], sinf_d[:])
        nc.scalar.dma_start(triu[:], triu_d[:])
        nc.scalar.dma_start(ones_col[:], ones_col_d[:])
        nc.scalar.dma_start(ident[:], ident_d[:])
        nc.scalar.dma_start(negcap[:], negcap_d[:])
        load_wq_sp8((0, 1))

        # per-chunk persistent tiles, filled as the pipeline progresses
        KT = {}    # sc -> [128, 512] bf16   (k^T, d on partitions)
        VN = {}    # sc -> [128, 512] bf16   (v natural, k on partitions)
        QT = {}    # (sc, h) -> [128, 512] bf16
        AOT = {}   # (sc, h) -> [128, 512] bf16
        ht_tiles = {}

        def load_ht(sc, split=False):
            """Generator: issue ht-chunk DMAs, 8 per unit."""
            for hb0 in range(0, N_HB, 8):
                for hb in range(hb0, hb0 + 8):
                    t = htp.tile([128, SCW], BF16, tag="ht", name="ht")
                    eng = nc.scalar if (split and hb % 2) else nc.sync
                    eng.dma_start(t[:], ht_d[sc, hb])
                    ht_tiles[(sc, hb)] = t
                yield

        def rope_epilogue(sc, ob, ps):
            scs = slice(sc * SCW, (sc + 1) * SCW)
            rot = ropep.tile([128, SCW], F32, tag="rot", name="rot")
            nc.scalar.copy(rot[0:64, :], ps[64:128, :])
            nc.scalar.copy(rot[64:128, :], ps[0:64, :])
            t1 = ropep.tile([128, SCW], F32, tag="t1", name="t1")
            nc.vector.tensor_mul(t1[:], ps[:], cosf[:, scs])
            nc.vector.tensor_mul(rot[:], rot[:], sinf[:, scs])
            if ob < NQ:
                qt = qtp.tile([128, SCW], BF16, tag="qt", name="qt")
                QT[(sc, ob)] = qt
                nc.vector.tensor_add(qt[:], t1[:], rot[:])
            else:
                kt = ktp.tile([128, SCW], BF16, tag="kt", name="kt")
                KT[sc] = kt
                nc.vector.tensor_add(kt[:], t1[:], rot[:])

        def v_epilogue(sc, ps, tp_pool, tp_tag):
            vt = vtp.tile([128, SCW], BF16, tag="vt", name="vt")
            nc.vector.tensor_copy(vt[:], ps[:])
            vn = vnp.tile([128, SCW], BF16, tag="vn", name="vn")
            VN[sc] = vn
            for j in range(4):
                tps = tp_pool.tile([128, 128], BF16, tag=tp_tag, name="tps")
                nc.tensor.transpose(tps[:], vt[:, j * 128:(j + 1) * 128], ident[:])
                nc.vector.tensor_copy(vn[:, j * 128:(j + 1) * 128], tps[:])

        def qkv0_stream():
            """qkv(0), hb-major over ob pairs: consumes each ht tile as it
            arrives instead of waiting for the full chunk."""
            pairs = [(0, 1), (2, 3), (4, 5), (6, 7)]
            for p, (o1, o2) in enumerate(pairs):
                if p + 1 < 4:
                    for o in pairs[p + 1]:
                        load_wq_gp4((0, o))
                pool = [ps_a, ps_s][p % 2]
                wa, wb = wq_pref.pop((0, o1)), wq_pref.pop((0, o2))
                psA = pool.tile([128, SCW], F32, tag=["acc", "s"][p % 2], name="psA")
                psB = pool.tile([128, SCW], F32, tag=["acc", "s"][p % 2], name="psB")
                for hb in range(N_HB):
                    nc.tensor.matmul(psA[:], lhsT=wa[:, hb * 128:(hb + 1) * 128],
                                     rhs=ht_tiles[(0, hb)][:],
                                     start=(hb == 0), stop=(hb == N_HB - 1))
                    nc.tensor.matmul(psB[:], lhsT=wb[:, hb * 128:(hb + 1) * 128],
                                     rhs=ht_tiles[(0, hb)][:],
                                     start=(hb == 0), stop=(hb == N_HB - 1))
                rope_epilogue(0, o1, psA)
                if o2 < 7:
                    rope_epilogue(0, o2, psB)
                else:
                    v_epilogue(0, psB, ps_pv, 'pv')
            load_wq_gp4((1, OB_ORDER[0]))

        def qkv_stream(sc):
            """Generator: qkv projection + rope for chunk sc (1..3). Yields at
            boundaries where attention work may be interleaved."""
            for idx, ob in enumerate(OB_ORDER):
                if idx + 1 < N_OB:
                    load_wq_gp4((sc, OB_ORDER[idx + 1]))
                elif sc < 3:
                    load_wq_gp4((sc + 1, OB_ORDER[0]))
                yield
                w_sb = wq_pref.pop((sc, ob))
                ps = ps_a.tile([128, SCW], F32, tag="acc", name="acc")
                for hb0 in range(0, N_HB, 4):
                    for hb in range(hb0, hb0 + 4):
                        nc.tensor.matmul(
                            ps[:],
                            lhsT=w_sb[:, hb * 128:(hb + 1) * 128],
                            rhs=ht_tiles[(sc, hb)][:],
                            start=(hb == 0),
                            stop=(hb == N_HB - 1),
                        )
                    yield
                if ob <= NQ:
                    rope_epilogue(sc, ob, ps)
                else:
                    v_epilogue(sc, ps, ps_a, 'acc')
                yield

        def attn_stream(qc):
            """Generator: attention for q-chunk qc, all 6 heads. Score matmuls
            run LOOK iterations ahead of PV; normalization is deferred one
            head so recip/broadcast never block the vector engine's triu."""
            nkb = 4 * qc + 4
            iters = [(h, kb) for h in range(NQ) for kb in range(nkb)]
            n = len(iters)
            LOOK = 2
            state = {}
            pv_cur = {}
            oa_cur = {}
            pend = []   # deferred (pv, bc, h) normalizations

            def issue_score(idx):
                h, kb = iters[idx]
                qs = max(qc * SCW, kb * 128)
                off = qs - qc * SCW
                w = SCW - off
                sp = ps_s.tile([128, SCW], F32, tag="s", name="s")
                nc.tensor.matmul(
                    sp[:, :w],
                    lhsT=KT[kb // 4][:, (kb % 4) * 128:(kb % 4 + 1) * 128],
                    rhs=QT[(qc, h)][:, off:SCW],
                    start=True, stop=True,
                )
                tt = tpool.tile([128, SCW], F32, tag="t", name="t")
                nc.scalar.activation(tt[:, :w], sp[:, :w], AF.Tanh,
                                     scale=SCALE / SOFTCAP)
                pt = ppool.tile([128, SCW], BF16, tag="p", name="p")
                nc.scalar.activation(pt[:, :w], tt[:, :w], AF.Exp,
                                     scale=SOFTCAP, bias=negcap[:])
                if kb >= 4 * qc:
                    nc.vector.tensor_mul(pt[:, 0:128], pt[:, 0:128], triu[:])
                state[idx] = (pt, w, off)

            def flush_norm():
                pv, bc, h = pend.pop(0)
                at = aotp.tile([128, SCW], BF16, tag="aot", name="aot")
                AOT[(qc, h)] = at
                nc.vector.tensor_mul(at[:], pv[:], bc[:])

            def issue_pv(idx):
                h, kb = iters[idx]
                pt, w, off = state.pop(idx)
                if kb == 0:
                    pv_cur[h] = ps_pv.tile([128, SCW], F32, tag="pv", name="pv")
                    oa_cur[h] = ps_o.tile([1, SCW], F32, tag="oa", name="oa")
                pv, oa = pv_cur[h], oa_cur[h]
                nc.tensor.matmul(
                    pv[:, off:SCW],
                    lhsT=VN[kb // 4][:, (kb % 4) * 128:(kb % 4 + 1) * 128],
                    rhs=pt[:, :w],
                    start=(kb == 0), stop=(kb == nkb - 1),
                )
                nc.tensor.matmul(
                    oa[0:1, off:SCW],
                    lhsT=ones_col[:],
                    rhs=pt[:, :w],
                    start=(kb == 0), stop=(kb == nkb - 1),
                )
                if kb == nkb - 1:
                    rr = rpool.tile([1, SCW], F32, tag="r", name="r")
                    nc.vector.reciprocal(rr[:], oa[0:1, :])
                    bc = bpool.tile([128, SCW], F32, tag="bc", name="bc")
                    nc.gpsimd.partition_broadcast(bc[:], rr[:])
                    if pend:
                        flush_norm()
                    pend.append((pv, bc, h))

            for j in range(min(LOOK, n)):
                issue_score(j)
            for i in range(n):
                if i + LOOK < n:
                    issue_score(i + LOOK)
                yield
                issue_pv(i)
            while pend:
                flush_norm()

        wo_tiles = {}

        def load_wo(mc):
            wos = wop.tile([128, N_FB * SCW], BF16, tag="wo", name="wo")
            for fb in range(N_FB):
                nc.sync.dma_start(wos[:, fb * SCW:(fb + 1) * SCW], wo_d[mc, :, fb])
            wo_tiles[mc] = wos

        def oproj_stream(mc_order, sb_list, prefetch, copy_split, keep=()):
            """Generator: o_proj partial for the given s-blocks / mc order.
            `prefetch` lists wo loads to issue (mc -> issue before which step);
            `keep` mcs stay resident in wo_tiles for the next phase."""
            for step, mc in enumerate(mc_order):
                for pmc in prefetch.get(step, []):
                    load_wo(pmc)
                wos = wo_tiles[mc] if mc in keep else wo_tiles.pop(mc)
                yield
                for i, sb in enumerate(sb_list):
                    sc, j = sb // 4, sb % 4
                    op = ps_a.tile([128, SCW], F32, tag="acc", name="acc")
                    for fb in range(N_FB):
                        nc.tensor.matmul(
                            op[:],
                            lhsT=AOT[(sc, fb)][:, j * 128:(j + 1) * 128],
                            rhs=wos[:, fb * SCW:(fb + 1) * SCW],
                            start=(fb == 0), stop=(fb == N_FB - 1),
                        )
                    ot = outp.tile([128, SCW], BF16, tag="out", name="out")
                    if copy_split and i % 2 == 1:
                        nc.scalar.copy(ot[:], op[:])
                    else:
                        nc.vector.tensor_copy(ot[:], op[:])
                    nc.sync.dma_start(
                        out_d[sb * 128:(sb + 1) * 128, mc * SCW:(mc + 1) * SCW], ot[:])
                    yield

        def chain(*gens):
            for g in gens:
                yield from g

        def interleave(primary, filler, n_primary, n_filler):
            """Advance primary; between slots advance filler so both streams
            finish together (adaptive ratio). Drain filler at the end."""
            rem_p, rem_f = n_primary, n_filler
            acc = 0.0
            f_done = False
            for _ in primary:
                rem_p -= 1
                if not f_done:
                    acc += rem_f / max(rem_p, 1)
                    while acc >= 1.0 and not f_done:
                        try:
                            next(filler)
                            rem_f -= 1
                        except StopIteration:
                            f_done = True
                        acc -= 1.0
            if not f_done:
                for _ in filler:
                    pass

        # ---- phase A: qkv(0), hb-major ----
        for _ in load_ht(0, split=True):
            pass
        qkv0_stream()

        # ---- phases B0..B2: attn(sc) ⋈ [ht(sc+1) prefetch + qkv(sc+1)] ----
        for sc in range(3):
            if sc == 2:  # prefetch wo for phase C
                load_wo(0)
                load_wo(1)
                load_wo(2)
            primary = attn_stream(sc)
            filler = chain(load_ht(sc + 1), qkv_stream(sc + 1))
            n_primary = NQ * (4 * sc + 4)
            n_filler = 6 + 14 * N_OB
            interleave(primary, filler, n_primary, n_filler)

        # ---- phase C: attn(3) ⋈ o_proj rows 0..11 ----
        primary = attn_stream(3)
        prefetch = {m: [m + 3] for m in range(9)}  # keep 3-deep wo pipeline
        filler = oproj_stream(list(range(N_MC)), list(range(12)), prefetch,
                              copy_split=False, keep={8, 9, 10, 11})
        interleave(primary, filler, NQ * 16, N_MC * 13)

        # ---- phase D: o_proj rows 12..15; wo 8..11 still resident ----
        d_order = [8, 9, 10, 11, 7, 6, 5, 4, 3, 2, 1, 0]
        prefetch = {0: [7, 6], 1: [5], 2: [4], 3: [3], 4: [2], 5: [1], 6: [0]}
        for _ in oproj_stream(d_order, list(range(12, 16)), prefetch,
                              copy_split=True):
            pass

    nc.compile()
    return nc


def prep_inputs(positions, hidden_states, w_qkv, w_o):
    """Host-side shard + relayout. Returns per-core input maps."""
    bf = ml_dtypes.bfloat16
    pos = np.asarray(positions).astype(np.float32)
    hidden = np.ascontiguousarray(np.asarray(hidden_states, dtype=np.float32))
    w_qkv = np.asarray(w_qkv, dtype=np.float32)
    w_o = np.asarray(w_o, dtype=np.float32)

    # rope tables (neox): freqs [S, 64]
    inv_freq = 1.0 / (ROPE_THETA ** (np.arange(0, D, 2, dtype=np.float32) / D))
    freqs = pos[:, None] * inv_freq[None, :]
    cos = np.cos(freqs).T.astype(np.float32)   # [64, S]
    sin = np.sin(freqs).T.astype(np.float32)
    cosf = np.concatenate([cos, cos], axis=0).astype(bf)    # [128, S]
    sinf = np.concatenate([-sin, sin], axis=0).astype(bf)

    triu = np.triu(np.ones((128, 128), np.float32)).astype(bf)  # [k, q]: q >= k
    ones_col = np.ones((128, 1), np.float32).astype(bf)
    ident = np.eye(128, dtype=np.float32).astype(bf)

    # ht[sc, hb, p, c] = hidden[sc*512+c, hb*128+p]
    ht = np.ascontiguousarray(
        hidden.reshape(N_SC, SCW, N_HB, 128).transpose(0, 2, 3, 1)).astype(bf)

    in_maps = []
    for c in range(N_CORES):
        q_rows = w_qkv[c * NQ * D:(c + 1) * NQ * D]          # [768, 6144]
        k_rows = w_qkv[HID + c * D:HID + (c + 1) * D]        # [128, 6144]
        v_rows = w_qkv[HID + 8 * D + c * D:HID + 8 * D + (c + 1) * D]
        wq_c = np.concatenate([q_rows, k_rows, v_rows], axis=0)  # [1024, 6144]
        # wq[ob, p, hb, o] = wq_c[ob*128+o, hb*128+p]
        wq_arr = np.ascontiguousarray(
            wq_c.reshape(N_OB, 128, N_HB, 128).transpose(0, 3, 2, 1)).astype(bf)
        wo_c = (w_o[:, c * NQ * D:(c + 1) * NQ * D] * ATTN_MULT).T  # [768, 6144]
        # wo[mc, p, fb, m] = wo_c[fb*128+p, mc*512+m]
        wo_arr = np.ascontiguousarray(
            wo_c.reshape(N_FB, 128, N_MC, SCW).transpose(2, 1, 0, 3)).astype(bf)
        in_maps.append({
            "ht": ht, "wq": wq_arr, "wo": wo_arr,
            "cosf": cosf, "sinf": sinf, "triu": triu,
            "ones_col": ones_col, "ident": ident,
            "negcap": np.full((128, 1), -SOFTCAP, np.float32),
        })
    return in_maps


_NC_CACHE = None


def _get_nc():
    global _NC_CACHE
    if _NC_CACHE is None:
        _NC_CACHE = build_nc()
    return _NC_CACHE


def kernel(positions, hidden_states, w_qkv, w_o, _trace=False, _trace_kwargs=None):
    nc = _get_nc()
    in_maps = prep_inputs(positions, hidden_states, w_qkv, w_o)
    res = run_bass_kernel_spmd(nc, in_maps, list(range(N_CORES)),
                               trace=_trace, **(_trace_kwargs or {}))
    out = np.zeros((S, HID), np.float32)
    for c in range(N_CORES):
        out += np.asarray(res.results[c]["out"]).astype(np.float32)
    out = out.astype(np.asarray(hidden_states).dtype)
    kernel.last_results = res
    return out


# revision 9
# speedup vs baseline: 1.0396x; 1.0396x over previous
"""Grok1-style GQA attention (S=2048, H=6144, 48 Q heads / 8 KV heads, rope,
softcap-30, causal) as a Bass/Tile kernel sharded over 8 NeuronCores.

Sharding: tensor-parallel across heads. Core c owns Q heads 6c..6c+5 and KV
head c. Each core computes its qkv projection slice, rope, causal softcap
attention for its 6 Q heads against its single KV head, and a partial
o_proj (its 768 columns of w_o). The host sums the 8 partial outputs.

Key numerics trick: softcap bounds scores to [-30, 30], so softmax is
computed as exp(30*tanh(s/30) - 30) with a *constant* bias — no running max.

Schedule: fully software-pipelined so the tensor engine never idles.
  A : qkv(0), hb-major over ob pairs so matmuls start as ht tiles land
  B0: attn(0) interleaved with qkv(1) matmuls   (+ ht(1) prefetch)
  B1: attn(1) interleaved with qkv(2)           (+ ht(2) prefetch)
  B2: attn(2) interleaved with qkv(3)           (+ ht(3) prefetch + wo(0..2))
  C : attn(3) interleaved with o_proj rows 0..11
  D : o_proj rows 12..15 (wo tiles for mc 8..11 still resident from C)
Within attention, score matmuls run 2 iterations ahead of the PV/rowsum
matmuls so the tanh->exp scalar chain never stalls the in-order PE queue;
per-head softmax normalization is deferred one head so the reciprocal/
broadcast chain never blocks the vector engine's triu masking.

Layouts (host-prepped, all transposed so the contraction dim is on SBUF
partitions):
  ht   [4,48,128,512] bf16  : ht[sc,hb,p,c] = hidden[sc*512+c, hb*128+p]
  wq   [8,128,48,128] bf16  : wq[ob,p,hb,o] = w_qkv_core[ob*128+o, hb*128+p]
  wo   [12,128,6,512] bf16  : wo[mc,p,fb,m] = (w_o[:,core]*MULT).T[fb*128+p, mc*512+m]
  cosf/sinf [128,2048] bf16 : duplicated/sign-flipped rope tables (neox)
  triu [128,128] bf16       : triu[k,q] = 1 if q >= k else 0
"""

import sys, os
import numpy as np

sys.path.insert(0, "/opt/trn_rl_repo")

import ml_dtypes

import concourse.bass as bass
import concourse.mybir as mybir
import concourse.tile as tile
from concourse import bacc
from concourse.bass_utils import run_bass_kernel_spmd

F32 = mybir.dt.float32
BF16 = mybir.dt.bfloat16
AF = mybir.ActivationFunctionType

S = 2048
HID = 6144
D = 128
NQ = 6          # q heads per core
N_CORES = 8
SCALE = D ** -0.5
SOFTCAP = 30.0
ATTN_MULT = 0.08838834764831845
ROPE_THETA = 10000.0

N_SC = 4        # s-chunks of 512
SCW = 512
N_HB = 48       # hidden 128-blocks
N_OB = 8        # output 128-blocks per core (6 Q | 1 K | 1 V)
N_MC = 12       # o_proj 512-col chunks
N_SB = 16       # s 128-blocks
N_FB = 6        # per-core o_proj feature 128-blocks (768/128)

OB_ORDER = [6, 7, 0, 1, 2, 3, 4, 5]   # K,V first so next phase never waits


def build_nc():
    nc = bacc.Bacc("TRN2", target_bir_lowering=False, debug=False, num_devices=N_CORES)

    ht_d = nc.dram_tensor("ht", [N_SC, N_HB, 128, SCW], BF16, kind="ExternalInput").ap()
    wq_d = nc.dram_tensor("wq", [N_OB, 128, N_HB, 128], BF16, kind="ExternalInput").ap()
    wo_d = nc.dram_tensor("wo", [N_MC, 128, N_FB, SCW], BF16, kind="ExternalInput").ap()
    cosf_d = nc.dram_tensor("cosf", [128, S], BF16, kind="ExternalInput").ap()
    sinf_d = nc.dram_tensor("sinf", [128, S], BF16, kind="ExternalInput").ap()
    triu_d = nc.dram_tensor("triu", [128, 128], BF16, kind="ExternalInput").ap()
    ones_col_d = nc.dram_tensor("ones_col", [128, 1], BF16, kind="ExternalInput").ap()
    ident_d = nc.dram_tensor("ident", [128, 128], BF16, kind="ExternalInput").ap()
    negcap_d = nc.dram_tensor("negcap", [128, 1], F32, kind="ExternalInput").ap()
    out_d = nc.dram_tensor("out", [S, HID], BF16, kind="ExternalOutput").ap()

    from contextlib import ExitStack
    with tile.TileContext(nc) as tc, ExitStack() as ctx:
        const = ctx.enter_context(tc.tile_pool(name="const", bufs=1))
        ktp = ctx.enter_context(tc.tile_pool(name="ktp", bufs=4))
        vnp = ctx.enter_context(tc.tile_pool(name="vnp", bufs=4))
        aotp = ctx.enter_context(tc.tile_pool(name="aotp", bufs=24))
        qtp = ctx.enter_context(tc.tile_pool(name="qtp", bufs=12))
        vtp = ctx.enter_context(tc.tile_pool(name="vtp", bufs=2))
        htp = ctx.enter_context(tc.tile_pool(name="htp", bufs=50))
        wqp = ctx.enter_context(tc.tile_pool(name="wqp", bufs=4))
        ropep = ctx.enter_context(tc.tile_pool(name="ropep", bufs=2))
        tpool = ctx.enter_context(tc.tile_pool(name="tpool", bufs=2))
        ppool = ctx.enter_context(tc.tile_pool(name="ppool", bufs=4))
        rpool = ctx.enter_context(tc.tile_pool(name="rpool", bufs=2))
        bpool = ctx.enter_context(tc.tile_pool(name="bpool", bufs=2))
        wop = ctx.enter_context(tc.tile_pool(name="wop", bufs=4))
        outp = ctx.enter_context(tc.tile_pool(name="outp", bufs=6))
        ps_a = ctx.enter_context(tc.tile_pool(name="ps_a", bufs=2, space=bass.MemorySpace.PSUM))
        ps_s = ctx.enter_context(tc.tile_pool(name="ps_s", bufs=2, space=bass.MemorySpace.PSUM))
        ps_pv = ctx.enter_context(tc.tile_pool(name="ps_pv", bufs=2, space=bass.MemorySpace.PSUM))
        ps_o = ctx.enter_context(tc.tile_pool(name="ps_o", bufs=2, space=bass.MemorySpace.PSUM))

        # ---- weight prefetch for qkv(0) first pair, via fast HWDGE ----
        wq_pref = {}

        def load_wq_gp4(sc_ob):
            ob = sc_ob[1]
            w_sb = wqp.tile([128, N_HB * 128], BF16, tag="wq", name="wq")
            for qd in range(4):
                nc.gpsimd.dma_start(
                    w_sb[:, qd * 12 * 128:(qd + 1) * 12 * 128],
                    wq_d[ob, :, qd * 12:(qd + 1) * 12])
            wq_pref[sc_ob] = w_sb

        load_wq_gp4((0, 6))
        load_wq_gp4((0, 7))

        cosf = const.tile([128, S], BF16, tag="cosf", name="cosf")
        sinf = const.tile([128, S], BF16, tag="sinf", name="sinf")
        triu = const.tile([128, 128], BF16, tag="triu", name="triu")
        ones_col = const.tile([128, 1], BF16, tag="ones_col", name="ones_col")
        ident = const.tile([128, 128], BF16, tag="ident", name="ident")
        negcap = const.tile([128, 1], F32, tag="negcap", name="negcap")
        nc.scalar.dma_start(cosf[:], cosf_d[:])
        nc.scalar.dma_start(sinf[:], sinf_d[:])
        nc.scalar.dma_start(triu[:], triu_d[:])
        nc.scalar.dma_start(ones_col[:], ones_col_d[:])
        nc.scalar.dma_start(ident[:], ident_d[:])
        nc.scalar.dma_start(negcap[:], negcap_d[:])

        # per-chunk persistent tiles, filled as the pipeline progresses
        KT = {}    # sc -> [128, 512] bf16   (k^T, d on partitions)
        VN = {}    # sc -> [128, 512] bf16   (v natural, k on partitions)
        QT = {}    # (sc, h) -> [128, 512] bf16
        AOT = {}   # (sc, h) -> [128, 512] bf16
        ht_tiles = {}

        def load_ht(sc, split=False):
            """Generator: issue ht-chunk DMAs, 8 per unit."""
            for hb0 in range(0, N_HB, 8):
                for hb in range(hb0, hb0 + 8):
                    t = htp.tile([128, SCW], BF16, tag="ht", name="ht")
                    eng = nc.scalar if (split and hb % 2) else nc.sync
                    eng.dma_start(t[:], ht_d[sc, hb])
                    ht_tiles[(sc, hb)] = t
                yield

        def rope_epilogue(sc, ob, ps):
            scs = slice(sc * SCW, (sc + 1) * SCW)
            rot = ropep.tile([128, SCW], F32, tag="rot", name="rot")
            nc.scalar.copy(rot[0:64, :], ps[64:128, :])
            nc.scalar.copy(rot[64:128, :], ps[0:64, :])
            t1 = ropep.tile([128, SCW], F32, tag="t1", name="t1")
            nc.vector.tensor_mul(t1[:], ps[:], cosf[:, scs])
            nc.vector.tensor_mul(rot[:], rot[:], sinf[:, scs])
            if ob < NQ:
                qt = qtp.tile([128, SCW], BF16, tag="qt", name="qt")
                QT[(sc, ob)] = qt
                nc.vector.tensor_add(qt[:], t1[:], rot[:])
            else:
                kt = ktp.tile([128, SCW], BF16, tag="kt", name="kt")
                KT[sc] = kt
                nc.vector.tensor_add(kt[:], t1[:], rot[:])

        def v_epilogue(sc, ps, tp_pool, tp_tag):
            vt = vtp.tile([128, SCW], BF16, tag="vt", name="vt")
            nc.vector.tensor_copy(vt[:], ps[:])
            vn = vnp.tile([128, SCW], BF16, tag="vn", name="vn")
            VN[sc] = vn
            for j in range(4):
                tps = tp_pool.tile([128, 128], BF16, tag=tp_tag, name="tps")
                nc.tensor.transpose(tps[:], vt[:, j * 128:(j + 1) * 128], ident[:])
                nc.vector.tensor_copy(vn[:, j * 128:(j + 1) * 128], tps[:])

        def qkv0_pair(o1, o2, pool, tag, next_pair):
            """Generator: one hb-major pass of qkv(0) over an ob pair."""
            if next_pair:
                for o in next_pair:
                    load_wq_gp4((0, o))
            wa, wb = wq_pref.pop((0, o1)), wq_pref.pop((0, o2))
            psA = pool.tile([128, SCW], F32, tag=tag, name="psA")
            psB = pool.tile([128, SCW], F32, tag=tag, name="psB")
            for hb0 in range(0, N_HB, 4):
                for hb in range(hb0, hb0 + 4):
                    nc.tensor.matmul(psA[:], lhsT=wa[:, hb * 128:(hb + 1) * 128],
                                     rhs=ht_tiles[(0, hb)][:],
                                     start=(hb == 0), stop=(hb == N_HB - 1))
                    nc.tensor.matmul(psB[:], lhsT=wb[:, hb * 128:(hb + 1) * 128],
                                     rhs=ht_tiles[(0, hb)][:],
                                     start=(hb == 0), stop=(hb == N_HB - 1))
                yield
            if o1 == 6:
                rope_epilogue(0, o1, psA)
                v_epilogue(0, psB, ps_pv, 'pv')
            else:
                rope_epilogue(0, o1, psA)
                rope_epilogue(0, o2, psB)
            yield

        def qkv_stream(sc):
            """Generator: qkv projection + rope for chunk sc (1..3). Yields at
            boundaries where attention work may be interleaved."""
            for idx, ob in enumerate(OB_ORDER):
                if idx + 1 < N_OB:
                    load_wq_gp4((sc, OB_ORDER[idx + 1]))
                elif sc < 3:
                    load_wq_gp4((sc + 1, OB_ORDER[0]))
                yield
                w_sb = wq_pref.pop((sc, ob))
                ps = ps_a.tile([128, SCW], F32, tag="acc", name="acc")
                for hb0 in range(0, N_HB, 4):
                    for hb in range(hb0, hb0 + 4):
                        nc.tensor.matmul(
                            ps[:],
                            lhsT=w_sb[:, hb * 128:(hb + 1) * 128],
                            rhs=ht_tiles[(sc, hb)][:],
                            start=(hb == 0),
                            stop=(hb == N_HB - 1),
                        )
                    yield
                if ob <= NQ:
                    rope_epilogue(sc, ob, ps)
                else:
                    v_epilogue(sc, ps, ps_a, 'acc')
                yield

        def attn_stream(qc):
            """Generator: attention for q-chunk qc, all 6 heads. Score matmuls
            run LOOK iterations ahead of PV; normalization is deferred one
            head so recip/broadcast never block the vector engine's triu."""
            nkb = 4 * qc + 4
            iters = [(h, kb) for h in range(NQ) for kb in range(nkb)]
            n = len(iters)
            LOOK = 2
            state = {}
            pv_cur = {}
            oa_cur = {}
            pend = []   # deferred (pv, bc, h) normalizations

            def issue_score(idx):
                h, kb = iters[idx]
                qs = max(qc * SCW, kb * 128)
                off = qs - qc * SCW
                w = SCW - off
                sp = ps_s.tile([128, SCW], F32, tag="s", name="s")
                nc.tensor.matmul(
                    sp[:, :w],
                    lhsT=KT[kb // 4][:, (kb % 4) * 128:(kb % 4 + 1) * 128],
                    rhs=QT[(qc, h)][:, off:SCW],
                    start=True, stop=True,
                )
                tt = tpool.tile([128, SCW], F32, tag="t", name="t")
                nc.scalar.activation(tt[:, :w], sp[:, :w], AF.Tanh,
                                     scale=SCALE / SOFTCAP)
                pt = ppool.tile([128, SCW], BF16, tag="p", name="p")
                nc.scalar.activation(pt[:, :w], tt[:, :w], AF.Exp,
                                     scale=SOFTCAP, bias=negcap[:])
                if kb >= 4 * qc:
                    nc.vector.tensor_mul(pt[:, 0:128], pt[:, 0:128], triu[:])
                state[idx] = (pt, w, off)

            def flush_norm():
                pv, bc, h = pend.pop(0)
                at = aotp.tile([128, SCW], BF16, tag="aot", name="aot")
                AOT[(qc, h)] = at
                nc.vector.tensor_mul(at[:], pv[:], bc[:])

            def issue_pv(idx):
                h, kb = iters[idx]
                pt, w, off = state.pop(idx)
                if kb == 0:
                    pv_cur[h] = ps_pv.tile([128, SCW], F32, tag="pv", name="pv")
                    oa_cur[h] = ps_o.tile([1, SCW], F32, tag="oa", name="oa")
                if kb == 1 and pend:
                    flush_norm()
                pv, oa = pv_cur[h], oa_cur[h]
                nc.tensor.matmul(
                    pv[:, off:SCW],
                    lhsT=VN[kb // 4][:, (kb % 4) * 128:(kb % 4 + 1) * 128],
                    rhs=pt[:, :w],
                    start=(kb == 0), stop=(kb == nkb - 1),
                )
                nc.tensor.matmul(
                    oa[0:1, off:SCW],
                    lhsT=ones_col[:],
                    rhs=pt[:, :w],
                    start=(kb == 0), stop=(kb == nkb - 1),
                )
                if kb == nkb - 1:
                    rr = rpool.tile([1, SCW], F32, tag="r", name="r")
                    nc.vector.reciprocal(rr[:], oa[0:1, :])
                    bc = bpool.tile([128, SCW], F32, tag="bc", name="bc")
                    nc.gpsimd.partition_broadcast(bc[:], rr[:])
                    pend.append((pv, bc, h))

            for j in range(min(LOOK, n)):
                issue_score(j)
            for i in range(n):
                if i + LOOK < n:
                    issue_score(i + LOOK)
                yield
                issue_pv(i)
            while pend:
                flush_norm()

        wo_tiles = {}

        def load_wo(mc):
            wos = wop.tile([128, N_FB * SCW], BF16, tag="wo", name="wo")
            for fb in range(N_FB):
                nc.sync.dma_start(wos[:, fb * SCW:(fb + 1) * SCW], wo_d[mc, :, fb])
            wo_tiles[mc] = wos

        def oproj_stream(mc_order, sb_list, prefetch, copy_split, keep=()):
            """Generator: o_proj partial for the given s-blocks / mc order.
            `prefetch` lists wo loads to issue (mc -> issue before which step);
            `keep` mcs stay resident in wo_tiles for the next phase."""
            for step, mc in enumerate(mc_order):
                for pmc in prefetch.get(step, []):
                    load_wo(pmc)
                wos = wo_tiles[mc] if mc in keep else wo_tiles.pop(mc)
                yield
                for i, sb in enumerate(sb_list):
                    sc, j = sb // 4, sb % 4
                    op = ps_a.tile([128, SCW], F32, tag="acc", name="acc")
                    for fb in range(N_FB):
                        nc.tensor.matmul(
                            op[:],
                            lhsT=AOT[(sc, fb)][:, j * 128:(j + 1) * 128],
                            rhs=wos[:, fb * SCW:(fb + 1) * SCW],
                            start=(fb == 0), stop=(fb == N_FB - 1),
                        )
                    ot = outp.tile([128, SCW], BF16, tag="out", name="out")
                    if copy_split and i % 2 == 1:
                        nc.scalar.copy(ot[:], op[:])
                    else:
                        nc.vector.tensor_copy(ot[:], op[:])
                    nc.sync.dma_start(
                        out_d[sb * 128:(sb + 1) * 128, mc * SCW:(mc + 1) * SCW], ot[:])
                    yield

        def chain(*gens):
            for g in gens:
                yield from g

        def interleave(primary, filler, n_primary, n_filler, reserve=0,
                       ratio=None, drain=True):
            """Advance primary; between slots advance filler so both streams
            finish together (adaptive, or fixed `ratio`). Keep `reserve`
            filler units unexecuted; drain (or hand back) the remainder."""
            rem_p, rem_f = n_primary, n_filler
            acc = 0.0
            f_done = False
            for _ in primary:
                rem_p -= 1
                if not f_done:
                    acc += ratio if ratio is not None else rem_f / max(rem_p, 1)
                    while acc >= 1.0 and not f_done and rem_f > reserve:
                        try:
                            next(filler)
                            rem_f -= 1
                        except StopIteration:
                            f_done = True
                        acc -= 1.0
            if drain and not f_done:
                for _ in filler:
                    pass
            return filler if not f_done else None

        def wo_loader(mcs):
            for mc in mcs:
                load_wo(mc)
                yield

        # ---- phase A: qkv(0), hb-major ob pairs, KV first ----
        for _ in load_ht(0, split=True):
            pass
        for _ in qkv0_pair(6, 7, ps_a, "acc", (0, 1)):
            pass
        for _ in qkv0_pair(0, 1, ps_s, "s", (2, 3)):
            pass
        for _ in qkv0_pair(2, 3, ps_s, "s", (4, 5)):
            pass
        load_wq_gp4((1, OB_ORDER[0]))
        carry = qkv0_pair(4, 5, ps_a, "acc", None)  # runs inside B0

        # ---- phases B0..B2: attn(sc) ⋈ [carry + ht(sc+1) + qkv(sc+1)] ----
        for sc in range(3):
            primary = attn_stream(sc)
            parts = [carry] if carry is not None else []
            parts += [load_ht(sc + 1), qkv_stream(sc + 1)]
            if sc == 2:  # stage wo(0) early, wo(1,2) in the reserved tail
                parts = parts[:2] + [wo_loader([0])] + parts[2:] + [wo_loader([1, 2])]
            n_carry = 13 if carry is not None else 0
            filler = chain(*parts)
            n_primary = NQ * (4 * sc + 4)
            n_filler = n_carry + 6 + 14 * N_OB + (3 if sc == 2 else 0)
            carry = interleave(primary, filler, n_primary, n_filler,
                               reserve=14)
        if carry is not None:
            for _ in carry:
                pass

        # ---- phase C: attn(3) ⋈ o_proj rows 0..11 (primary retires early) ----
        primary = attn_stream(3)
        prefetch = {m: [m + 3] for m in range(9)}  # keep 3-deep wo pipeline
        filler = oproj_stream(list(range(N_MC)), list(range(12)), prefetch,
                              copy_split=False, keep={8, 9, 10, 11})
        interleave(primary, filler, NQ * 16, N_MC * 13, ratio=1.0)

        # ---- phase D: o_proj rows 12..15; wo 8..11 still resident ----
        d_order = [8, 9, 10, 11, 7, 6, 5, 4, 3, 2, 1, 0]
        prefetch = {0: [7, 6], 1: [5], 2: [4], 3: [3], 4: [2], 5: [1], 6: [0]}
        for _ in oproj_stream(d_order, list(range(12, 16)), prefetch,
                              copy_split=True):
            pass

    nc.compile()
    return nc


def prep_inputs(positions, hidden_states, w_qkv, w_o):
    """Host-side shard + relayout. Returns per-core input maps."""
    bf = ml_dtypes.bfloat16
    pos = np.asarray(positions).astype(np.float32)
    hidden = np.ascontiguousarray(np.asarray(hidden_states, dtype=np.float32))
    w_qkv = np.asarray(w_qkv, dtype=np.float32)
    w_o = np.asarray(w_o, dtype=np.float32)

    # rope tables (neox): freqs [S, 64]
    inv_freq = 1.0 / (ROPE_THETA ** (np.arange(0, D, 2, dtype=np.float32) / D))
    freqs = pos[:, None] * inv_freq[None, :]
    cos = np.cos(freqs).T.astype(np.float32)   # [64, S]
    sin = np.sin(freqs).T.astype(np.float32)
    cosf = np.concatenate([cos, cos], axis=0).astype(bf)    # [128, S]
    sinf = np.concatenate([-sin, sin], axis=0).astype(bf)

    triu = np.triu(np.ones((128, 128), np.float32)).astype(bf)  # [k, q]: q >= k
    ones_col = np.ones((128, 1), np.float32).astype(bf)
    ident = np.eye(128, dtype=np.float32).astype(bf)

    # ht[sc, hb, p, c] = hidden[sc*512+c, hb*128+p]
    ht = np.ascontiguousarray(
        hidden.reshape(N_SC, SCW, N_HB, 128).transpose(0, 2, 3, 1)).astype(bf)

    in_maps = []
    for c in range(N_CORES):
        q_rows = w_qkv[c * NQ * D:(c + 1) * NQ * D]          # [768, 6144]
        k_rows = w_qkv[HID + c * D:HID + (c + 1) * D]        # [128, 6144]
        v_rows = w_qkv[HID + 8 * D + c * D:HID + 8 * D + (c + 1) * D]
        wq_c = np.concatenate([q_rows, k_rows, v_rows], axis=0)  # [1024, 6144]
        # wq[ob, p, hb, o] = wq_c[ob*128+o, hb*128+p]
        wq_arr = np.ascontiguousarray(
            wq_c.reshape(N_OB, 128, N_HB, 128).transpose(0, 3, 2, 1)).astype(bf)
        wo_c = (w_o[:, c * NQ * D:(c + 1) * NQ * D] * ATTN_MULT).T  # [768, 6144]
        # wo[mc, p, fb, m] = wo_c[fb*128+p, mc*512+m]
        wo_arr = np.ascontiguousarray(
            wo_c.reshape(N_FB, 128, N_MC, SCW).transpose(2, 1, 0, 3)).astype(bf)
        in_maps.append({
            "ht": ht, "wq": wq_arr, "wo": wo_arr,
            "cosf": cosf, "sinf": sinf, "triu": triu,
            "ones_col": ones_col, "ident": ident,
            "negcap": np.full((128, 1), -SOFTCAP, np.float32),
        })
    return in_maps


_NC_CACHE = None


def _get_nc():
    global _NC_CACHE
    if _NC_CACHE is None:
        _NC_CACHE = build_nc()
    return _NC_CACHE


def kernel(positions, hidden_states, w_qkv, w_o, _trace=False, _trace_kwargs=None):
    nc = _get_nc()
    in_maps = prep_inputs(positions, hidden_states, w_qkv, w_o)
    res = run_bass_kernel_spmd(nc, in_maps, list(range(N_CORES)),
                               trace=_trace, **(_trace_kwargs or {}))
    out = np.zeros((S, HID), np.float32)
    for c in range(N_CORES):
        out += np.asarray(res.results[c]["out"]).astype(np.float32)
    out = out.astype(np.asarray(hidden_states).dtype)
    kernel.last_results = res
    return out


# revision 11
# speedup vs baseline: 1.0447x; 1.0049x over previous
"""Grok1-style GQA attention (S=2048, H=6144, 48 Q heads / 8 KV heads, rope,
softcap-30, causal) as a Bass/Tile kernel sharded over 8 NeuronCores.

Sharding: tensor-parallel across heads. Core c owns Q heads 6c..6c+5 and KV
head c. Each core computes its qkv projection slice, rope, causal softcap
attention for its 6 Q heads against its single KV head, and a partial
o_proj (its 768 columns of w_o). The host sums the 8 partial outputs.

Key numerics trick: softcap bounds scores to [-30, 30], so softmax is
computed as exp(30*tanh(s/30) - 30) with a *constant* bias — no running max.

Schedule: fully software-pipelined so the tensor engine never idles.
  A : qkv(0), hb-major over ob pairs so matmuls start as ht tiles land
  B0: attn(0) interleaved with qkv(1) matmuls   (+ ht(1) prefetch)
  B1: attn(1) interleaved with qkv(2)           (+ ht(2) prefetch)
  B2: attn(2) interleaved with qkv(3)           (+ ht(3) prefetch + wo(0..2))
  C : attn(3) interleaved with o_proj rows 0..11
  D : o_proj rows 12..15 (wo tiles for mc 8..11 still resident from C)
Within attention, score matmuls run 2 iterations ahead of the PV/rowsum
matmuls so the tanh->exp scalar chain never stalls the in-order PE queue;
per-head softmax normalization is deferred one head so the reciprocal/
broadcast chain never blocks the vector engine's triu masking.

Layouts (host-prepped, all transposed so the contraction dim is on SBUF
partitions):
  ht   [4,48,128,512] bf16  : ht[sc,hb,p,c] = hidden[sc*512+c, hb*128+p]
  wq   [8,128,48,128] bf16  : wq[ob,p,hb,o] = w_qkv_core[ob*128+o, hb*128+p]
  wo   [12,128,6,512] bf16  : wo[mc,p,fb,m] = (w_o[:,core]*MULT).T[fb*128+p, mc*512+m]
  cosf/sinf [128,2048] bf16 : duplicated/sign-flipped rope tables (neox)
  triu [128,128] bf16       : triu[k,q] = 1 if q >= k else 0
"""

import sys, os
import numpy as np

sys.path.insert(0, "/opt/trn_rl_repo")

import ml_dtypes

import concourse.bass as bass
import concourse.mybir as mybir
import concourse.tile as tile
from concourse import bacc
from concourse.bass_utils import run_bass_kernel_spmd

F32 = mybir.dt.float32
BF16 = mybir.dt.bfloat16
AF = mybir.ActivationFunctionType

S = 2048
HID = 6144
D = 128
NQ = 6          # q heads per core
N_CORES = 8
SCALE = D ** -0.5
SOFTCAP = 30.0
ATTN_MULT = 0.08838834764831845
ROPE_THETA = 10000.0

N_SC = 4        # s-chunks of 512
SCW = 512
N_HB = 48       # hidden 128-blocks
N_OB = 8        # output 128-blocks per core (6 Q | 1 K | 1 V)
N_MC = 12       # o_proj 512-col chunks
N_SB = 16       # s 128-blocks
N_FB = 6        # per-core o_proj feature 128-blocks (768/128)

OB_ORDER = [6, 7, 0, 1, 2, 3, 4, 5]   # K,V first so next phase never waits


def build_nc():
    nc = bacc.Bacc("TRN2", target_bir_lowering=False, debug=False, num_devices=N_CORES)

    ht_d = nc.dram_tensor("ht", [N_SC, N_HB, 128, SCW], BF16, kind="ExternalInput").ap()
    wq_d = nc.dram_tensor("wq", [N_OB, 128, N_HB, 128], BF16, kind="ExternalInput").ap()
    wo_d = nc.dram_tensor("wo", [N_MC, 128, N_FB, SCW], BF16, kind="ExternalInput").ap()
    cosf_d = nc.dram_tensor("cosf", [128, S], BF16, kind="ExternalInput").ap()
    sinf_d = nc.dram_tensor("sinf", [128, S], BF16, kind="ExternalInput").ap()
    triu_d = nc.dram_tensor("triu", [128, 128], BF16, kind="ExternalInput").ap()
    ones_col_d = nc.dram_tensor("ones_col", [128, 1], BF16, kind="ExternalInput").ap()
    ident_d = nc.dram_tensor("ident", [128, 128], BF16, kind="ExternalInput").ap()
    negcap_d = nc.dram_tensor("negcap", [128, 1], F32, kind="ExternalInput").ap()
    out_d = nc.dram_tensor("out", [S, HID], BF16, kind="ExternalOutput").ap()

    from contextlib import ExitStack
    with tile.TileContext(nc) as tc, ExitStack() as ctx:
        const = ctx.enter_context(tc.tile_pool(name="const", bufs=1))
        ktp = ctx.enter_context(tc.tile_pool(name="ktp", bufs=4))
        vnp = ctx.enter_context(tc.tile_pool(name="vnp", bufs=4))
        aotp = ctx.enter_context(tc.tile_pool(name="aotp", bufs=24))
        qtp = ctx.enter_context(tc.tile_pool(name="qtp", bufs=12))
        vtp = ctx.enter_context(tc.tile_pool(name="vtp", bufs=2))
        htp = ctx.enter_context(tc.tile_pool(name="htp", bufs=49))
        wqp = ctx.enter_context(tc.tile_pool(name="wqp", bufs=4))
        ropep = ctx.enter_context(tc.tile_pool(name="ropep", bufs=2))
        tpool = ctx.enter_context(tc.tile_pool(name="tpool", bufs=2))
        ppool = ctx.enter_context(tc.tile_pool(name="ppool", bufs=4))
        rpool = ctx.enter_context(tc.tile_pool(name="rpool", bufs=2))
        bpool = ctx.enter_context(tc.tile_pool(name="bpool", bufs=2))
        wop = ctx.enter_context(tc.tile_pool(name="wop", bufs=4))
        outp = ctx.enter_context(tc.tile_pool(name="outp", bufs=8))
        ps_a = ctx.enter_context(tc.tile_pool(name="ps_a", bufs=2, space=bass.MemorySpace.PSUM))
        ps_s = ctx.enter_context(tc.tile_pool(name="ps_s", bufs=2, space=bass.MemorySpace.PSUM))
        ps_pv = ctx.enter_context(tc.tile_pool(name="ps_pv", bufs=2, space=bass.MemorySpace.PSUM))
        ps_o = ctx.enter_context(tc.tile_pool(name="ps_o", bufs=2, space=bass.MemorySpace.PSUM))

        # ---- weight prefetch for qkv(0) first pair, via fast HWDGE ----
        wq_pref = {}

        def load_wq_gp4(sc_ob):
            ob = sc_ob[1]
            w_sb = wqp.tile([128, N_HB * 128], BF16, tag="wq", name="wq")
            for qd in range(4):
                nc.gpsimd.dma_start(
                    w_sb[:, qd * 12 * 128:(qd + 1) * 12 * 128],
                    wq_d[ob, :, qd * 12:(qd + 1) * 12])
            wq_pref[sc_ob] = w_sb

        load_wq_gp4((0, 6))
        load_wq_gp4((0, 7))

        cosf = const.tile([128, S], BF16, tag="cosf", name="cosf")
        sinf = const.tile([128, S], BF16, tag="sinf", name="sinf")
        triu = const.tile([128, 128], BF16, tag="triu", name="triu")
        ones_col = const.tile([128, 1], BF16, tag="ones_col", name="ones_col")
        ident = const.tile([128, 128], BF16, tag="ident", name="ident")
        negcap = const.tile([128, 1], F32, tag="negcap", name="negcap")
        nc.scalar.dma_start(cosf[:], cosf_d[:])
        nc.scalar.dma_start(sinf[:], sinf_d[:])
        nc.scalar.dma_start(triu[:], triu_d[:])
        nc.scalar.dma_start(ones_col[:], ones_col_d[:])
        nc.scalar.dma_start(ident[:], ident_d[:])
        nc.scalar.dma_start(negcap[:], negcap_d[:])

        # per-chunk persistent tiles, filled as the pipeline progresses
        KT = {}    # sc -> [128, 512] bf16   (k^T, d on partitions)
        VN = {}    # sc -> [128, 512] bf16   (v natural, k on partitions)
        QT = {}    # (sc, h) -> [128, 512] bf16
        AOT = {}   # (sc, h) -> [128, 512] bf16
        ht_tiles = {}

        def load_ht(sc, split=False):
            """Generator: issue ht-chunk DMAs, 4 per unit."""
            for hb0 in range(0, N_HB, 4):
                for hb in range(hb0, hb0 + 4):
                    t = htp.tile([128, SCW], BF16, tag="ht", name="ht")
                    eng = nc.scalar if (split and hb % 2) else nc.sync
                    eng.dma_start(t[:], ht_d[sc, hb])
                    ht_tiles[(sc, hb)] = t
                yield

        def rope_epilogue(sc, ob, ps):
            scs = slice(sc * SCW, (sc + 1) * SCW)
            rot = ropep.tile([128, SCW], F32, tag="rot", name="rot")
            nc.scalar.copy(rot[0:64, :], ps[64:128, :])
            nc.scalar.copy(rot[64:128, :], ps[0:64, :])
            t1 = ropep.tile([128, SCW], F32, tag="t1", name="t1")
            nc.vector.tensor_mul(t1[:], ps[:], cosf[:, scs])
            nc.vector.tensor_mul(rot[:], rot[:], sinf[:, scs])
            if ob < NQ:
                qt = qtp.tile([128, SCW], BF16, tag="qt", name="qt")
                QT[(sc, ob)] = qt
                nc.vector.tensor_add(qt[:], t1[:], rot[:])
            else:
                kt = ktp.tile([128, SCW], BF16, tag="kt", name="kt")
                KT[sc] = kt
                nc.vector.tensor_add(kt[:], t1[:], rot[:])

        def v_epilogue(sc, ps, tp_pool, tp_tag):
            vt = vtp.tile([128, SCW], BF16, tag="vt", name="vt")
            nc.vector.tensor_copy(vt[:], ps[:])
            vn = vnp.tile([128, SCW], BF16, tag="vn", name="vn")
            VN[sc] = vn
            for j in range(4):
                tps = tp_pool.tile([128, 128], BF16, tag=tp_tag, name="tps")
                nc.tensor.transpose(tps[:], vt[:, j * 128:(j + 1) * 128], ident[:])
                nc.vector.tensor_copy(vn[:, j * 128:(j + 1) * 128], tps[:])

        def qkv0_pair(o1, o2, pool, tag, next_pair):
            """Generator: one hb-major pass of qkv(0) over an ob pair. The
            next pair's weights load mid-pass so early DMA bandwidth goes to
            the ht tiles this pass is consuming."""
            wa, wb = wq_pref.pop((0, o1)), wq_pref.pop((0, o2))
            psA = pool.tile([128, SCW], F32, tag=tag, name="psA")
            psB = pool.tile([128, SCW], F32, tag=tag, name="psB")
            for hb0 in range(0, N_HB, 4):
                if hb0 == 24 and next_pair:
                    for o in next_pair:
                        load_wq_gp4((0, o))
                for hb in range(hb0, hb0 + 4):
                    nc.tensor.matmul(psA[:], lhsT=wa[:, hb * 128:(hb + 1) * 128],
                                     rhs=ht_tiles[(0, hb)][:],
                                     start=(hb == 0), stop=(hb == N_HB - 1))
                    nc.tensor.matmul(psB[:], lhsT=wb[:, hb * 128:(hb + 1) * 128],
                                     rhs=ht_tiles[(0, hb)][:],
                                     start=(hb == 0), stop=(hb == N_HB - 1))
                yield
            if o1 == 6:
                rope_epilogue(0, o1, psA)
                v_epilogue(0, psB, ps_pv, 'pv')
            else:
                rope_epilogue(0, o1, psA)
                rope_epilogue(0, o2, psB)
            yield

        def qkv_stream(sc):
            """Generator: qkv projection + rope for chunk sc (1..3). Yields at
            boundaries where attention work may be interleaved."""
            for idx, ob in enumerate(OB_ORDER):
                if idx + 1 < N_OB:
                    load_wq_gp4((sc, OB_ORDER[idx + 1]))
                elif sc < 3:
                    load_wq_gp4((sc + 1, OB_ORDER[0]))
                yield
                w_sb = wq_pref.pop((sc, ob))
                ps = ps_a.tile([128, SCW], F32, tag="acc", name="acc")
                for hb0 in range(0, N_HB, 4):
                    for hb in range(hb0, hb0 + 4):
                        nc.tensor.matmul(
                            ps[:],
                            lhsT=w_sb[:, hb * 128:(hb + 1) * 128],
                            rhs=ht_tiles[(sc, hb)][:],
                            start=(hb == 0),
                            stop=(hb == N_HB - 1),
                        )
                    yield
                if ob <= NQ:
                    rope_epilogue(sc, ob, ps)
                else:
                    v_epilogue(sc, ps, ps_a, 'acc')
                yield

        def attn_stream(qc):
            """Generator: attention for q-chunk qc, all 6 heads. Score matmuls
            run LOOK iterations ahead of PV; normalization is deferred one
            head so recip/broadcast never block the vector engine's triu."""
            nkb = 4 * qc + 4
            iters = [(h, kb) for h in range(NQ) for kb in range(nkb)]
            n = len(iters)
            LOOK = 2
            state = {}
            pv_cur = {}
            oa_cur = {}
            pend = []   # deferred (pv, bc, h) normalizations

            def issue_score(idx):
                h, kb = iters[idx]
                qs = max(qc * SCW, kb * 128)
                off = qs - qc * SCW
                w = SCW - off
                sp = ps_s.tile([128, SCW], F32, tag="s", name="s")
                nc.tensor.matmul(
                    sp[:, :w],
                    lhsT=KT[kb // 4][:, (kb % 4) * 128:(kb % 4 + 1) * 128],
                    rhs=QT[(qc, h)][:, off:SCW],
                    start=True, stop=True,
                )
                tt = tpool.tile([128, SCW], F32, tag="t", name="t")
                nc.scalar.activation(tt[:, :w], sp[:, :w], AF.Tanh,
                                     scale=SCALE / SOFTCAP)
                pt = ppool.tile([128, SCW], BF16, tag="p", name="p")
                nc.scalar.activation(pt[:, :w], tt[:, :w], AF.Exp,
                                     scale=SOFTCAP, bias=negcap[:])
                if kb >= 4 * qc:
                    nc.vector.tensor_mul(pt[:, 0:128], pt[:, 0:128], triu[:])
                state[idx] = (pt, w, off)

            def flush_norm():
                pv, bc, h = pend.pop(0)
                at = aotp.tile([128, SCW], BF16, tag="aot", name="aot")
                AOT[(qc, h)] = at
                nc.vector.tensor_mul(at[:], pv[:], bc[:])

            def issue_pv(idx):
                h, kb = iters[idx]
                pt, w, off = state.pop(idx)
                if kb == 0:
                    pv_cur[h] = ps_pv.tile([128, SCW], F32, tag="pv", name="pv")
                    oa_cur[h] = ps_o.tile([1, SCW], F32, tag="oa", name="oa")
                if kb == 1 and pend:
                    flush_norm()
                pv, oa = pv_cur[h], oa_cur[h]
                nc.tensor.matmul(
                    pv[:, off:SCW],
                    lhsT=VN[kb // 4][:, (kb % 4) * 128:(kb % 4 + 1) * 128],
                    rhs=pt[:, :w],
                    start=(kb == 0), stop=(kb == nkb - 1),
                )
                nc.tensor.matmul(
                    oa[0:1, off:SCW],
                    lhsT=ones_col[:],
                    rhs=pt[:, :w],
                    start=(kb == 0), stop=(kb == nkb - 1),
                )
                if kb == nkb - 1:
                    rr = rpool.tile([1, SCW], F32, tag="r", name="r")
                    nc.vector.reciprocal(rr[:], oa[0:1, :])
                    bc = bpool.tile([128, SCW], F32, tag="bc", name="bc")
                    nc.gpsimd.partition_broadcast(bc[:], rr[:])
                    pend.append((pv, bc, h))

            for j in range(min(LOOK, n)):
                issue_score(j)
            for i in range(n):
                if i + LOOK < n:
                    issue_score(i + LOOK)
                yield
                issue_pv(i)
            while pend:
                flush_norm()

        wo_tiles = {}

        def load_wo(mc):
            wos = wop.tile([128, N_FB * SCW], BF16, tag="wo", name="wo")
            for fb in range(N_FB):
                nc.sync.dma_start(wos[:, fb * SCW:(fb + 1) * SCW], wo_d[mc, :, fb])
            wo_tiles[mc] = wos

        def oproj_stream(mc_order, sb_list, prefetch, copy_split, keep=()):
            """Generator: o_proj partial for the given s-blocks / mc order.
            `prefetch` lists wo loads to issue (mc -> issue before which step);
            `keep` mcs stay resident in wo_tiles for the next phase."""
            for step, mc in enumerate(mc_order):
                for pmc in prefetch.get(step, []):
                    load_wo(pmc)
                wos = wo_tiles[mc] if mc in keep else wo_tiles.pop(mc)
                yield
                for i, sb in enumerate(sb_list):
                    sc, j = sb // 4, sb % 4
                    op = ps_a.tile([128, SCW], F32, tag="acc", name="acc")
                    for fb in range(N_FB):
                        nc.tensor.matmul(
                            op[:],
                            lhsT=AOT[(sc, fb)][:, j * 128:(j + 1) * 128],
                            rhs=wos[:, fb * SCW:(fb + 1) * SCW],
                            start=(fb == 0), stop=(fb == N_FB - 1),
                        )
                    ot = outp.tile([128, SCW], BF16, tag="out", name="out")
                    if copy_split and i % 2 == 1:
                        nc.scalar.copy(ot[:], op[:])
                    else:
                        nc.vector.tensor_copy(ot[:], op[:])
                    nc.sync.dma_start(
                        out_d[sb * 128:(sb + 1) * 128, mc * SCW:(mc + 1) * SCW], ot[:])
                    yield

        def chain(*gens):
            for g in gens:
                yield from g

        def interleave(primary, filler, n_primary, n_filler, reserve=0,
                       ratio=None, drain=True):
            """Advance primary; between slots advance filler so both streams
            finish together (adaptive, or fixed `ratio`). Keep `reserve`
            filler units unexecuted; drain (or hand back) the remainder."""
            rem_p, rem_f = n_primary, n_filler
            acc = 0.0
            f_done = False
            for _ in primary:
                rem_p -= 1
                if not f_done:
                    acc += ratio if ratio is not None else rem_f / max(rem_p, 1)
                    while acc >= 1.0 and not f_done and rem_f > reserve:
                        try:
                            next(filler)
                            rem_f -= 1
                        except StopIteration:
                            f_done = True
                        acc -= 1.0
            if drain and not f_done:
                for _ in filler:
                    pass
            return filler if not f_done else None

        def wo_loader(mcs):
            for mc in mcs:
                load_wo(mc)
                yield

        # ---- phase A: qkv(0), hb-major ob pairs, KV first ----
        for _ in load_ht(0, split=True):
            pass
        for _ in qkv0_pair(6, 7, ps_a, "acc", (0, 1)):
            pass
        for _ in qkv0_pair(0, 1, ps_s, "s", (2, 3)):
            pass
        for _ in qkv0_pair(2, 3, ps_s, "s", (4, 5)):
            pass
        load_wq_gp4((1, OB_ORDER[0]))
        carry = qkv0_pair(4, 5, ps_a, "acc", None)  # runs inside B0

        # ---- phases B0..B2: attn(sc) ⋈ [carry + ht(sc+1) + qkv(sc+1)] ----
        for sc in range(3):
            primary = attn_stream(sc)
            parts = [carry] if carry is not None else []
            parts += [load_ht(sc + 1), qkv_stream(sc + 1)]
            if sc == 2:  # stage wo(0) early, wo(1,2) in the reserved tail
                parts = parts[:2] + [wo_loader([0])] + parts[2:] + [wo_loader([1, 2])]
            n_carry = 13 if carry is not None else 0
            filler = chain(*parts)
            n_primary = NQ * (4 * sc + 4)
            n_filler = n_carry + 12 + 14 * N_OB + (3 if sc == 2 else 0)
            carry = interleave(primary, filler, n_primary, n_filler,
                               reserve=14)
        if carry is not None:
            for _ in carry:
                pass

        # ---- phase C: attn(3) ⋈ o_proj rows 0..11 (primary retires early) ----
        primary = attn_stream(3)
        prefetch = {m: [m + 3] for m in range(9)}  # keep 3-deep wo pipeline
        filler = oproj_stream(list(range(N_MC)), list(range(12)), prefetch,
                              copy_split=False, keep={8, 9, 10, 11})
        interleave(primary, filler, NQ * 16, N_MC * 13, ratio=1.0)

        # ---- phase D: o_proj rows 12..15; wo 8..11 still resident ----
        d_order = [8, 9, 10, 11, 7, 6, 5, 4, 3, 2, 1, 0]
        prefetch = {0: [7, 6], 1: [5], 2: [4], 3: [3], 4: [2], 5: [1], 6: [0]}
        for _ in oproj_stream(d_order, list(range(12, 16)), prefetch,
                              copy_split=True):
            pass

    nc.compile()
    return nc


def prep_inputs(positions, hidden_states, w_qkv, w_o):
    """Host-side shard + relayout. Returns per-core input maps."""
    bf = ml_dtypes.bfloat16
    pos = np.asarray(positions).astype(np.float32)
    hidden = np.ascontiguousarray(np.asarray(hidden_states, dtype=np.float32))
    w_qkv = np.asarray(w_qkv, dtype=np.float32)
    w_o = np.asarray(w_o, dtype=np.float32)

    # rope tables (neox): freqs [S, 64]
    inv_freq = 1.0 / (ROPE_THETA ** (np.arange(0, D, 2, dtype=np.float32) / D))
    freqs = pos[:, None] * inv_freq[None, :]
    cos = np.cos(freqs).T.astype(np.float32)   # [64, S]
    sin = np.sin(freqs).T.astype(np.float32)
    cosf = np.concatenate([cos, cos], axis=0).astype(bf)    # [128, S]
    sinf = np.concatenate([-sin, sin], axis=0).astype(bf)

    triu = np.triu(np.ones((128, 128), np.float32)).astype(bf)  # [k, q]: q >= k
    ones_col = np.ones((128, 1), np.float32).astype(bf)
    ident = np.eye(128, dtype=np.float32).astype(bf)

    # ht[sc, hb, p, c] = hidden[sc*512+c, hb*128+p]
    ht = np.ascontiguousarray(
        hidden.reshape(N_SC, SCW, N_HB, 128).transpose(0, 2, 3, 1)).astype(bf)

    in_maps = []
    for c in range(N_CORES):
        q_rows = w_qkv[c * NQ * D:(c + 1) * NQ * D]          # [768, 6144]
        k_rows = w_qkv[HID + c * D:HID + (c + 1) * D]        # [128, 6144]
        v_rows = w_qkv[HID + 8 * D + c * D:HID + 8 * D + (c + 1) * D]
        wq_c = np.concatenate([q_rows, k_rows, v_rows], axis=0)  # [1024, 6144]
        # wq[ob, p, hb, o] = wq_c[ob*128+o, hb*128+p]
        wq_arr = np.ascontiguousarray(
            wq_c.reshape(N_OB, 128, N_HB, 128).transpose(0, 3, 2, 1)).astype(bf)
        wo_c = (w_o[:, c * NQ * D:(c + 1) * NQ * D] * ATTN_MULT).T  # [768, 6144]
        # wo[mc, p, fb, m] = wo_c[fb*128+p, mc*512+m]
        wo_arr = np.ascontiguousarray(
            wo_c.reshape(N_FB, 128, N_MC, SCW).transpose(2, 1, 0, 3)).astype(bf)
        in_maps.append({
            "ht": ht, "wq": wq_arr, "wo": wo_arr,
            "cosf": cosf, "sinf": sinf, "triu": triu,
            "ones_col": ones_col, "ident": ident,
            "negcap": np.full((128, 1), -SOFTCAP, np.float32),
        })
    return in_maps


_NC_CACHE = None


def _get_nc():
    global _NC_CACHE
    if _NC_CACHE is None:
        _NC_CACHE = build_nc()
    return _NC_CACHE


def kernel(positions, hidden_states, w_qkv, w_o, _trace=False, _trace_kwargs=None):
    nc = _get_nc()
    in_maps = prep_inputs(positions, hidden_states, w_qkv, w_o)
    res = run_bass_kernel_spmd(nc, in_maps, list(range(N_CORES)),
                               trace=_trace, **(_trace_kwargs or {}))
    out = np.zeros((S, HID), np.float32)
    for c in range(N_CORES):
        out += np.asarray(res.results[c]["out"]).astype(np.float32)
    out = out.astype(np.asarray(hidden_states).dtype)
    kernel.last_results = res
    return out
